# revision 25
# baseline (speedup 1.0000x reference)
"""GAT (3-layer) Trainium2 Bass kernel, 8-way node-sharded.

Self-contained: host preprocessing (graph partitioning, relabeling, edge
stream construction) + Bass/Tile kernel + gather/unshard.

Strategy:
  - Relabel nodes so core c owns new ids [c*NLOC, (c+1)*NLOC); blocks of 128
    dst nodes; per-block uniform chunk quotas (TE even-src + TO odd-src
    chunks of 128 edges each) with sentinel padding.
  - Layer-0 gather table is built LOCALLY on every core from the full
    (replicated, host-pre-transposed, bf16) x — no AllGather on layer 0.
  - Layers 1/2 tables: each core computes its slice (fused hf+attention
    matmul via a combined weight tile), sliced AllGather (nslice slices)
    fires as soon as each slice's staging groups are written, overlapping
    the collective with the remaining edge-phase compute.
  - Edge phase: dma_gather of per-edge rows (parity-split tables to fit
    int16 indices), exp(lrelu(s_src+s_dst)) per edge, features scaled by
    per-head ee, one-hot matmul on PE accumulates per-dst sums + softmax
    denominators in PSUM.
  - Finalize batched per group: normalize/affine/ELU on the whole group,
    then per-block PE transposes + one fused dense matmul producing the
    next layer's features AND attention scalars.
"""
import numpy as np
import ml_dtypes
from contextlib import ExitStack

import concourse.bacc as bacc
import concourse.bass as bass
import concourse.mybir as mybir
import concourse.tile as tile
from concourse.bass_utils import run_bass_kernel_spmd

P = 128
NCORES = 8
EPS_BN = 1e-5
NEG = -1e38
F32 = mybir.dt.float32
BF16 = mybir.dt.bfloat16
I16 = mybir.dt.int16
I8 = mybir.dt.int8
BF = ml_dtypes.bfloat16

# Full-problem constants (matches reference.py / spec.json)
N_FULL, E_FULL, F_IN, HID, HEADS, OUTD = 50000, 800000, 128, 64, 4, 2


# ----------------------------------------------------------------------------
# Host preprocessing
# ----------------------------------------------------------------------------

def preprocess(edge_index, n_nodes, nblk, te, to):
    """Graph partitioning + edge stream construction.

    Returns (new_id [n_nodes], plans per core). All integer index work only.
    """
    T = te + to
    nloc = nblk * P
    cap_e, cap_o = te * P, to * P
    src = np.concatenate([edge_index[0].astype(np.int64), np.arange(n_nodes)])
    dst = np.concatenate([edge_index[1].astype(np.int64), np.arange(n_nodes)])
    deg = np.bincount(dst, minlength=n_nodes)

    # nodes -> cores, balancing in-edge load
    order = np.argsort(-deg, kind="stable")
    cap_nodes = n_nodes // NCORES
    core_of = np.empty(n_nodes, np.int64)
    loads = np.zeros(NCORES, np.int64)
    counts = np.zeros(NCORES, np.int64)
    for n in order:
        avail = np.flatnonzero(counts < cap_nodes)
        c = avail[np.argmin(loads[avail])]
        core_of[n] = c
        loads[c] += deg[n]
        counts[c] += 1

    # per core: nodes -> blocks, balancing block load
    new_id = np.full(n_nodes, -1, np.int64)
    for c in range(NCORES):
        nodes = np.flatnonzero(core_of == c)
        nodes = nodes[np.argsort(-deg[nodes], kind="stable")]
        bload = np.zeros(nblk, np.int64)
        bcount = np.zeros(nblk, np.int64)
        slot_ctr = np.zeros(nblk, np.int64)
        for n in nodes:
            avail = np.flatnonzero(bcount < P)
            b = avail[np.argmin(bload[avail])]
            bload[b] += deg[n]
            bcount[b] += 1
            new_id[n] = c * nloc + b * P + slot_ctr[b]
            slot_ctr[b] += 1
        if bload.max() > T * P:
            raise RuntimeError(f"block overflow: {bload.max()} > {T*P}")

    nsrc = new_id[src]
    ndst = new_id[dst]
    ngb = NCORES * nblk

    # parity repair: per global block, even-src count <= cap_e, odd <= cap_o
    rng = np.random.default_rng(1)
    for _ in range(20000):
        gblk = ndst // P
        ec = np.bincount(gblk[(nsrc & 1) == 0], minlength=ngb)
        tc = np.bincount(gblk, minlength=ngb)
        bad = np.flatnonzero((ec > cap_e) | ((tc - ec) > cap_o))
        if not len(bad):
            break
        b = bad[0]
        par = 0 if ec[b] > cap_e else 1
        eidx = np.flatnonzero(gblk == b)
        cand = eidx[(nsrc[eidx] & 1) == par]
        sn = nsrc[cand[rng.integers(len(cand))]]
        sb = (sn // P) * P
        partners = np.arange(sb + (1 - par), sb + P, 2)
        pn = partners[rng.integers(len(partners))]
        for arr in (nsrc, ndst):
            ms, mp = arr == sn, arr == pn
            arr[ms] = pn
            arr[mp] = sn
        os_ = np.flatnonzero(new_id == sn)[0]
        op_ = np.flatnonzero(new_id == pn)[0]
        new_id[os_], new_id[op_] = pn, sn
    else:
        raise RuntimeError("parity repair failed")

    # per-core streams in tile order:
    # group-major; within a group of G blocks: [all even chunks | all odd chunks]
    plans = []
    for c in range(NCORES):
        sel = (ndst >= c * nloc) & (ndst < (c + 1) * nloc)
        es, ed = nsrc[sel], ndst[sel] - c * nloc
        b = ed // P
        par = (es & 1).astype(np.int64)
        o = np.lexsort((es, par, b))
        es, ed, b, par = es[o], ed[o], b[o], par[o]
        # per-block, per-parity slot ranges (block-major natural order first)
        idx_e = np.full((nblk, cap_e), -1, np.int64)
        ds_e = np.zeros((nblk, cap_e), np.int64)
        idx_o = np.full((nblk, cap_o), -1, np.int64)
        ds_o = np.zeros((nblk, cap_o), np.int64)
        for blk in range(nblk):
            m = b == blk
            for pp, idx_a, ds_a, cap in ((0, idx_e, ds_e, cap_e), (1, idx_o, ds_o, cap_o)):
                mm = m & (par == pp)
                k = int(mm.sum())
                assert k <= cap
                idx_a[blk, :k] = es[mm]
                ds_a[blk, :k] = ed[mm] % P
        plans.append(dict(idx_e=idx_e, ds_e=ds_e, idx_o=idx_o, ds_o=ds_o))
    return new_id, plans


def _wrap_idx(a):
    """[k] int -> wrapped [128, k/16] int16 (16-partition wrap, replicated x8)."""
    w = a.reshape(-1, 16).T.astype(np.int16)
    return np.ascontiguousarray(np.tile(w, (8, 1)))


# ----------------------------------------------------------------------------
# Bass kernel builder
# ----------------------------------------------------------------------------

def build_kernel(cfg):
    """cfg: dict(nblk, te, to, g, n_nodes). Returns nc."""
    nblk, te, to, G = cfg["nblk"], cfg["te"], cfg["to"], cfg["g"]
    skip_ag = cfg.get("skip_ag", False)
    skip_gather = cfg.get("skip_gather", False)
    skip_fg = cfg.get("skip_fg", False)       # skip feature gathers only
    reps = cfg.get("reps", 1)
    nslice = cfg.get("nslice", 1)
    dst8 = cfg.get("dst8", 1)
    T = te + to
    nloc = nblk * P
    npair_c = nloc // 2              # pairs per core (incl dummies)
    npair = NCORES * npair_c         # rows per parity table (excl sentinel)
    groups = [(i, min(i + G, nblk)) for i in range(0, nblk, G)]
    cuts = [round(i * len(groups) / nslice) for i in range(nslice + 1)]
    # slice s covers groups [cuts[s], cuts[s+1]) -> blocks [sb0, sb1)
    slc = []
    for s in range(nslice):
        sb0 = groups[cuts[s]][0] if cuts[s] < len(groups) else nblk
        sb1 = groups[cuts[s + 1] - 1][1] if cuts[s + 1] > cuts[s] else sb0
        slc.append((sb0, sb1))
    end_group_slice = {cuts[s + 1] - 1: s for s in range(nslice) if cuts[s + 1] > cuts[s]}
    # per-block slice bounds (for direct tab0 row addressing)
    blk_b0 = [0] * nblk
    blk_b1 = [nblk] * nblk
    for s in range(nslice):
        if cuts[s + 1] > cuts[s]:
            sb0 = groups[cuts[s]][0]
            sb1 = groups[cuts[s + 1] - 1][1]
            for b in range(sb0, sb1):
                blk_b0[b], blk_b1[b] = sb0, sb1

    ROW1 = HEADS * (HID + 1)                 # 260 bf16 cols of payload
    TBLW = 384                                # L0/L1 row width (bf16 cols)
    TBLW2 = 128                               # L2 row width
    SOFF = 260                                # s_dst f32 at bf16 col 260 (byte 520)
    SOFF2 = 66                                # L2: s at col 66 (byte 132)
    NCH = [ROW1, ROW1, HID + 1]               # matmul rhs widths per layer
    ROWW = [TBLW, TBLW, TBLW2]
    SOFFS = [SOFF, SOFF, SOFF2]
    NH = [HEADS, HEADS, 1]

    nq = int(cfg.get("nq", 4))
    # compact AG staging: stage/transfer only the used row prefix per layer
    cag = cfg.get("cag", 0)
    CW = [TBLW, 276, 72] if cag else [TBLW, TBLW, TBLW2]
    nc = bacc.Bacc("TRN2", num_devices=NCORES, num_swdge_queues=nq)
    dt = nc.dram_tensor

    # ---- inputs
    xT_in = dt("xT", [F_IN, NCORES * nloc], BF16, kind="ExternalInput")
    xTo_in = dt("xTo", [F_IN, nloc], BF16, kind="ExternalInput")
    W_in = [dt(f"W{l}", [F_IN if l == 0 else HEADS * HID, (HEADS if l < 2 else 1) * HID], F32, kind="ExternalInput") for l in range(3)]
    Wc_in = dt("Wc", [HID, OUTD], F32, kind="ExternalInput")
    A_in = [dt(f"A{l}", [(HEADS if l < 2 else 1) * HID, 2 * (HEADS if l < 2 else 1)], F32, kind="ExternalInput") for l in range(3)]
    bn_in = [dt(f"bn{l}", [5, (HEADS if l < 2 else 1) * HID], F32, kind="ExternalInput") for l in range(3)]  # rows: b,g,bt,m,v
    bc_in = dt("bc", [1, OUTD], F32, kind="ExternalInput")
    ident_in = dt("ident", [P, P], F32, kind="ExternalInput")
    iota_in = dt("iota", [P, P], BF16, kind="ExternalInput")

    n_e = te * P
    n_o = to * P
    idx_e_in = dt("idx_e", [P, nblk * n_e // 16], I16, kind="ExternalInput")
    idx_o_in = dt("idx_o", [P, nblk * n_o // 16], I16, kind="ExternalInput")
    dslot_in = dt("dslot", [P, nblk * T], BF16, kind="ExternalInput")
    DST_DT = I8 if dst8 else BF16
    dslotTr_in = dt("dslotTr", [P, nblk * T * P], DST_DT, kind="ExternalInput")

    y_out = dt("y", [nloc, OUTD], F32, kind="ExternalOutput")

    if cfg.get("noop"):
        # identical external I/O, near-zero work: for warm-wall dispatch baseline
        with tile.TileContext(nc) as tc:
            with tc.tile_pool(name="np0", bufs=1) as pool:
                z = pool.tile([P, nblk, OUTD], F32)
                nc.vector.memset(z[:], 0.0)
                nc.sync.dma_start(y_out[:].rearrange("(g q) w -> q g w", g=nblk), z[:])
        nc.compile()
        return nc

    # ---- internal DRAM: tables + staging
    tab_e = [dt(f"tab{l}e", [npair + 1, ROWW[l]], BF16, kind="Internal", addr_space="Shared") for l in range(3)]
    tab_o = [dt(f"tab{l}o", [npair + 1, ROWW[l]], BF16, kind="Internal", addr_space="Shared") for l in range(3)]
    stg_e = [dt(f"stg{l}e", [npair_c, CW[l]], BF16, kind="Internal") for l in (1, 2)]
    stg_o = [dt(f"stg{l}o", [npair_c, CW[l]], BF16, kind="Internal") for l in (1, 2)]
    stg_e = [None] + stg_e
    stg_o = [None] + stg_o

    rg = [list(range(NCORES))]

    with tile.TileContext(nc) as tc, ExitStack() as ctx:
        cst = ctx.enter_context(tc.tile_pool(name="cst", bufs=1))
        wrk = ctx.enter_context(tc.tile_pool(name="wrk", bufs=2))
        wrk1 = ctx.enter_context(tc.tile_pool(name="wrk1", bufs=1))
        pk = ctx.enter_context(tc.tile_pool(name="pk", bufs=2))
        ps = ctx.enter_context(tc.tile_pool(name="ps", bufs=2, space="PSUM"))
        ps1 = ctx.enter_context(tc.tile_pool(name="ps1", bufs=1, space="PSUM"))

        # ---- constants to SBUF
        ident = cst.tile([P, P], F32)
        nc.sync.dma_start(ident[:], ident_in[:])
        ones1 = cst.tile([1, P], BF16)
        nc.vector.memset(ones1[:], 1.0)

        def bcast_row(dst_sb, row_ap, w):
            # dst_sb[p, :w] = row_ap[0, :w] for all partitions, via K=1 matmul
            bp = ps1.tile([P, 512], F32, tag="tph")
            rb = wrk.tile([1, 512], BF16, tag="rbf")
            nc.vector.tensor_copy(rb[:, :w], row_ap)
            nc.tensor.matmul(bp[:, :w], ones1[:], rb[:, :w], start=True, stop=True)
            nc.vector.tensor_copy(dst_sb, bp[:, :w])
        iota = cst.tile([P, P], BF16)
        nc.sync.dma_start(iota[:], iota_in[:])
        dslot_sb = cst.tile([P, nblk * T], BF16)
        nc.sync.dma_start(dslot_sb[:], dslot_in[:])
        # iotaT[p, 0] = p for the transposed one-hot build
        iotaT_i = cst.tile([P, 1], mybir.dt.int32)
        nc.gpsimd.iota(iotaT_i[:], [[1, 1]], channel_multiplier=1)
        iotaT = cst.tile([P, 1], DST_DT)
        nc.vector.tensor_copy(iotaT[:], iotaT_i[:])
        # per-block s_dst of current/next layer (ping-pong across layers)
        sdst_a = cst.tile([P, nblk, HEADS], BF16, tag="sdst0")
        sdst_b = cst.tile([P, nblk, HEADS], BF16, tag="sdst1")
        sdst_t = [sdst_a, sdst_b]
        bc_rep = cst.tile([P, OUTD], F32)
        bcl = wrk.tile([1, OUTD], F32, tag="bcl")
        nc.sync.dma_start(bcl[:], bc_in[:])
        bcast_row(bc_rep[:], bcl[0:1, :], OUTD)

        # per-layer weight tiles (bf16): fused [W | A_src | A_dst] chunks + BN affine
        WAt = []    # [K-chunk [128, 2*nh] bf16]
        Wcmb = []   # [K-chunk [128, fout + 2*nh] bf16] fused dense+attention
        Wct = None
        sc_rep = []
        sh_rep = []
        for l in range(3):
            fin = F_IN if l == 0 else HEADS * HID
            fout = (HEADS if l < 2 else 1) * HID
            nh = NH[l]
            nkin = fin // P
            # load W f32, cast to bf16 chunk tiles
            wf = wrk1.tile([P, (fin // P) * fout], F32, tag="wload")
            nc.sync.dma_start(wf[:].rearrange("p (k f) -> p k f", k=nkin),
                              W_in[l].rearrange("(k p) f -> p k f", p=P))
            wb = cst.tile([P, (fin // P) * fout], BF16, tag=f"wb{l}")
            nc.vector.tensor_copy(wb[:], wf[:])

            # WT stored as f32 sbuf [fout rows over ceil chunks][fin]
            nchT = (fout + P - 1) // P
            wT = cst.tile([P, nchT * fin], F32, tag=f"wT{l}")
            for ki in range(nkin):          # W row chunk (fin dim)
                for kj in range(nchT):      # W col chunk (fout dim)
                    cw = min(P, fout - kj * P)
                    tp = ps1.tile([P, P], F32, tag="tph")
                    nc.tensor.transpose(tp[:cw, :P], wf[:, ki * fout + kj * P: ki * fout + kj * P + cw], ident[:])
                    nc.vector.tensor_copy(wT[:cw, kj * fin + ki * P: kj * fin + (ki + 1) * P], tp[:cw, :P])
            # A -> sbuf
            fo_p = min(P, fout)
            af = wrk.tile([P, nchT * 2 * nh], F32, tag="aload")
            if fout >= P:
                nc.sync.dma_start(af[:].rearrange("p (k f) -> p k f", k=nchT),
                                  A_in[l].rearrange("(k p) f -> p k f", p=P))
            else:
                nc.sync.dma_start(af[:fo_p, 0:2 * nh], A_in[l][:])
            ab = af
            # WA [fin, 2nh] = sum_kj WT_chunk.T @ A_chunk  (bf16 for dense use)
            wab = cst.tile([P, nkin * 2 * nh], BF16, tag=f"wab{l}")
            for ki in range(nkin):
                wa_ps = ps1.tile([P, 2 * nh], F32, tag="sps")
                for kj in range(nchT):
                    cw = min(P, fout - kj * P)
                    nc.tensor.matmul(wa_ps[:],
                                     wT[:cw, kj * fin + ki * P: kj * fin + (ki + 1) * P],
                                     ab[:cw, kj * 2 * nh:(kj + 1) * 2 * nh],
                                     start=(kj == 0), stop=(kj == nchT - 1))
                nc.vector.tensor_copy(wab[:, ki * 2 * nh:(ki + 1) * 2 * nh], wa_ps[:])
            WAt.append([wab[:, k * 2 * nh:(k + 1) * 2 * nh] for k in range(nkin)])
            # fused [W chunk | WA chunk] per K-chunk
            cw_f = fout + 2 * nh
            wcmb = cst.tile([P, nkin * cw_f], BF16, tag=f"wcmb{l}")
            for k in range(nkin):
                nc.vector.tensor_copy(wcmb[:, k * cw_f:k * cw_f + fout], wb[:, k * fout:(k + 1) * fout])
                nc.vector.tensor_copy(wcmb[:, k * cw_f + fout:(k + 1) * cw_f], wab[:, k * 2 * nh:(k + 1) * 2 * nh])
            Wcmb.append([wcmb[:, k * cw_f:(k + 1) * cw_f] for k in range(nkin)])

            # BN affine: scale2 = g/sqrt(v+eps); shift2 = (b - m)*scale2 + bt
            bn = wrk1.tile([1, 5 * fout], F32, tag="bnload")
            nc.sync.dma_start(bn[:].rearrange("p (r f) -> p r f", r=5), bn_in[l][None, :, :])
            bnr = [bn[:, i * fout:(i + 1) * fout] for i in range(5)]  # b,g,bt,m,v
            sc1 = wrk.tile([1, fout], F32, tag="sc1")
            nc.vector.tensor_scalar(out=sc1[:], in0=bnr[4], scalar1=EPS_BN, scalar2=None, op0=mybir.AluOpType.add)
            nc.scalar.activation(sc1[:], sc1[:], mybir.ActivationFunctionType.Sqrt)
            rc = wrk.tile([1, fout], F32, tag="rc1")
            nc.vector.reciprocal(rc[:], sc1[:])
            nc.vector.tensor_tensor(out=rc[:], in0=rc[:], in1=bnr[1], op=mybir.AluOpType.mult)
            sh1 = wrk.tile([1, fout], F32, tag="sh1")
            nc.vector.tensor_tensor(out=sh1[:], in0=bnr[0], in1=bnr[3], op=mybir.AluOpType.subtract)
            nc.vector.tensor_tensor(out=sh1[:], in0=sh1[:], in1=rc[:], op=mybir.AluOpType.mult)
            nc.vector.tensor_tensor(out=sh1[:], in0=sh1[:], in1=bnr[2], op=mybir.AluOpType.add)
            screp = cst.tile([P, fout], F32, tag=f"screp{l}")
            bcast_row(screp[:], rc[0:1, :], fout)
            shrep = cst.tile([P, fout], F32, tag=f"shrep{l}")
            bcast_row(shrep[:], sh1[0:1, :], fout)
            sc_rep.append(screp)
            sh_rep.append(shrep)

        wcf = wrk.tile([HID, OUTD], F32, tag="wcl")
        nc.sync.dma_start(wcf[:], Wc_in[:])
        Wct = cst.tile([HID, OUTD], BF16)
        nc.vector.tensor_copy(Wct[:], wcf[:])

        # ---- sentinel rows: feats 0 (incl ones col), s slots NEG
        sent = cst.tile([1, TBLW], BF16)
        nc.vector.memset(sent[:], 0)
        sent32 = sent[:].bitcast(F32)
        nc.vector.memset(sent32[:, SOFF // 2: SOFF // 2 + 8], NEG)
        sent2 = cst.tile([1, TBLW2], BF16)
        nc.vector.memset(sent2[:], 0)
        nc.vector.memset(sent2[:].bitcast(F32)[:, SOFF2 // 2: SOFF2 // 2 + 2], NEG)
        for l in range(3):
            st = sent2 if l == 2 else sent
            nc.sync.dma_start(tab_e[l][npair:npair + 1, :], st[:])
            nc.sync.dma_start(tab_o[l][npair:npair + 1, :], st[:])

        # sliced AllGather: fire slice s of layer l as soon as its staging
        # groups are written, overlapping with remaining groups' compute
        def ag_slice(l, s):
            sb0, sb1 = slc[s]
            nr = (sb1 - sb0) * 64
            if nr == 0:
                return
            cw = CW[l]
            for stg, tab in ((stg_e[l], tab_e[l]), (stg_o[l], tab_o[l])):
                if skip_ag:
                    nc.sync.dma_start(tab[512 * sb0:512 * sb0 + nr, 0:cw],
                                      stg[64 * sb0:64 * sb1, :])
                else:
                    nc.gpsimd.collective_compute(
                        "AllGather", mybir.AluOpType.bypass,
                        ins=[stg[64 * sb0:64 * sb1, :]],
                        outs=[tab[512 * sb0:512 * sb0 + 8 * nr, 0:cw]],
                        replica_groups=rg)

        # ================= per-group batched finalize ====================
        def finalize_group(l, gacc, gn, g0, pack, sdst_nxt, pack_y):
            """gacc: SBUF [P, G, NCH[l]] f32 (per-head 64 feats + denom).
            Batched normalize/affine/ELU, then per-block transposes + fused
            dense matmul -> next-layer packed rows + s_dst (or y for l==2)."""
            nh = NH[l]
            fout = nh * HID
            gview = gacc[:, 0:gn, :].rearrange("p g (h c) -> p g h c", c=HID + 1)
            dn = wrk.tile([P, G, nh], F32, tag="dn")
            nc.vector.tensor_scalar(out=dn[:, 0:gn, :], in0=gview[:, :, :, HID], scalar1=1e-30, scalar2=None, op0=mybir.AluOpType.add)
            rd = wrk.tile([P, G, nh], F32, tag="rd")
            nc.vector.reciprocal(rd[:, 0:gn, :], dn[:, 0:gn, :])
            h = wrk.tile([P, G, fout], F32, tag="h")
            nc.vector.tensor_tensor(
                out=h[:, 0:gn, :].rearrange("p g (h c) -> p g h c", h=nh),
                in0=gview[:, :, :, 0:HID],
                in1=rd[:, 0:gn, :, None].to_broadcast([P, gn, nh, HID]),
                op=mybir.AluOpType.mult)
            nc.vector.tensor_tensor(out=h[:, 0:gn, :], in0=h[:, 0:gn, :],
                                    in1=sc_rep[l][:, None, 0:fout].to_broadcast([P, gn, fout]),
                                    op=mybir.AluOpType.mult)
            nc.vector.tensor_tensor(out=h[:, 0:gn, :], in0=h[:, 0:gn, :],
                                    in1=sh_rep[l][:, None, 0:fout].to_broadcast([P, gn, fout]),
                                    op=mybir.AluOpType.add)
            if l < 2:
                u = wrk1.tile([P, G, fout], F32, tag="elu_u")
                nc.vector.tensor_scalar(out=u[:, 0:gn, :], in0=h[:, 0:gn, :], scalar1=0.0, scalar2=None, op0=mybir.AluOpType.min)
                nc.scalar.activation(u[:, 0:gn, :], u[:, 0:gn, :], mybir.ActivationFunctionType.Exp)
                nc.vector.tensor_scalar(out=h[:, 0:gn, :], in0=h[:, 0:gn, :], scalar1=0.0, scalar2=-1.0, op0=mybir.AluOpType.max, op1=mybir.AluOpType.add)
                nc.vector.tensor_tensor(out=h[:, 0:gn, :], in0=h[:, 0:gn, :], in1=u[:, 0:gn, :], op=mybir.AluOpType.add)
            nkin = fout // P if fout >= P else 1
            for bi in range(gn):
                b = g0 + bi
                hT = wrk.tile([P, nkin * P], BF16, tag="hT")
                for k in range(nkin):
                    cw = min(P, fout - k * P)
                    tp = ps1.tile([P, P], F32, tag="tph")
                    nc.tensor.transpose(tp[:cw, :], h[:, bi, k * P:k * P + cw], ident[:])
                    nc.vector.tensor_copy(hT[:cw, k * P:(k + 1) * P], tp[:cw, :])
                if l == 2:
                    # head: y = h2 @ Wc + bc
                    yp = ps1.tile([P, OUTD], F32, tag="sps")
                    nc.tensor.matmul(yp[:], hT[:HID, 0:P], Wct[:], start=True, stop=True)
                    nc.vector.tensor_tensor(out=pack_y[:, bi, :], in0=yp[:], in1=bc_rep[:], op=mybir.AluOpType.add)
                    continue
                # fused dense: [hf_next | s_src | s_dst] = h @ [W | A] of layer l+1
                nl = l + 1
                nhn = NH[nl]
                fon = nhn * HID
                hfs = ps.tile([P, fon + 2 * nhn], F32, tag="hfps")
                for k in range(nkin):
                    nc.tensor.matmul(hfs[:], hT[:, k * P:(k + 1) * P], Wcmb[nl][k], start=(k == 0), stop=(k == nkin - 1))
                nc.vector.tensor_copy(
                    pack[:, bi, 0:nhn * (HID + 1)].rearrange("p (h c) -> p h c", h=nhn)[:, :, 0:HID],
                    hfs[:, 0:fon].rearrange("p (h c) -> p h c", h=nhn))
                pk32 = pack[:].bitcast(F32)
                so = SOFFS[nl] // 2
                nc.vector.tensor_copy(pk32[:, bi, so + nhn:so + 2 * nhn], hfs[:, fon:fon + nhn])
                nc.vector.tensor_copy(sdst_nxt[:, b, 0:nhn], hfs[:, fon + nhn:fon + 2 * nhn])

        def rep_body():
            # ================= L0 own-block s_dst =====================
            spsg = ps.tile([P, nblk, 2 * HEADS], F32, tag="sdps")
            for (g0, g1) in groups:
                gn = g1 - g0
                xtg = wrk.tile([P, G * P], BF16, tag="xtg")
                nc.sync.dma_start(xtg[:, 0:gn * P], xTo_in[:, g0 * P:g1 * P])
                for bi in range(gn):
                    nc.tensor.matmul(spsg[:, g0 + bi, :], xtg[:, bi * P:(bi + 1) * P],
                                     WAt[0][0], start=True, stop=True)
            nc.vector.tensor_copy(sdst_t[0][:, :, 0:HEADS], spsg[:, :, HEADS:2 * HEADS])

            # ====== L0 table: every core builds the FULL table locally ======
            for c in range(NCORES):
                for (g0, g1) in groups:
                    gn = g1 - g0
                    pack = pk.tile([P, G, TBLW], BF16, tag="pack0")
                    xtg = wrk.tile([P, G * P], BF16, tag="xtg")
                    nc.sync.dma_start(xtg[:, 0:gn * P],
                                      xT_in[:, c * nloc + g0 * P: c * nloc + g1 * P])
                    for bi in range(gn):
                        hfs = ps.tile([P, HEADS * HID + 2 * HEADS], F32, tag="hfps")
                        nc.tensor.matmul(hfs[:], xtg[:, bi * P:(bi + 1) * P], Wcmb[0][0],
                                         start=True, stop=True)
                        nc.vector.tensor_copy(
                            pack[:, bi, 0:HEADS * (HID + 1)].rearrange("p (h c) -> p h c", h=HEADS)[:, :, 0:HID],
                            hfs[:, 0:HEADS * HID].rearrange("p (h c) -> p h c", h=HEADS))
                        pk32 = pack[:].bitcast(F32)
                        so = SOFF // 2
                        nc.vector.tensor_copy(pk32[:, bi, so + HEADS:so + 2 * HEADS],
                                              hfs[:, HEADS * HID:HEADS * HID + HEADS])
                    on = pack[:, 0:gn, 0:HEADS * (HID + 1)].rearrange("p g (h c) -> p g h c", h=HEADS)[:, :, :, HID:HID + 1]
                    nc.vector.memset(on, 1.0)
                    b0, b1 = blk_b0[g0], blk_b1[g0]
                    base = 512 * b0 + c * 64 * (b1 - b0) + 64 * (g0 - b0)
                    for par, tab in ((0, tab_e[0]), (1, tab_o[0])):
                        nc.sync.dma_start(
                            tab[base:base + gn * 64, :].rearrange("(g q) w -> q g w", g=gn),
                            pack[par::2, 0:gn, :])

            # ================= layers =====================
            # Software-pipelined: stage A (gathers, one-hot builds, s_dst
            # matmuls) of group g+1 is emitted BEFORE stage B (edge math,
            # aggregation, finalize) of group g, so each engine FIFO always
            # holds ready work from the other group while a chain stalls.
            for l in range(3):
                nh = NH[l]
                roww = ROWW[l]
                nch_w = NCH[l]
                soff = SOFFS[l]
                sdst_cur = sdst_t[l % 2]
                sdst_nxt = sdst_t[(l + 1) % 2]

                def stage_a(gi):
                    g0, g1 = groups[gi]
                    gn = g1 - g0
                    nche = gn * te       # even chunks in group
                    ncho = gn * to
                    ncht = nche + ncho
                    # chunk order in tiles: [even chunks | odd chunks]
                    ixe = wrk.tile([P, nche * 8], I16, tag="ixe")
                    nc.sync.dma_start(ixe[:], idx_e_in[:, g0 * n_e // 16:(g0 * n_e + nche * P) // 16])
                    ixo = wrk.tile([P, ncho * 8], I16, tag="ixo")
                    nc.sync.dma_start(ixo[:], idx_o_in[:, g0 * n_o // 16:(g0 * n_o + ncho * P) // 16])

                    gw = bool(cfg.get("gw256")) and l < 2   # timing-only probe: fetch 512B of each 768B row
                    roww_g = 256 if gw else roww
                    g = wrk.tile([P, G * T, roww_g], BF16, tag="g")
                    if skip_gather or skip_fg:
                        nc.vector.memset(g[:, 0:ncht, :], 0)
                    else:
                        spkt = bool(cfg.get("spkt", False))
                        nqh = max(1, nq // 2)
                        for par, tabl, ix, c0, ncp, q0 in (
                                (0, tab_e[l], ixe, 0, nche, 0),
                                (1, tab_o[l], ixo, nche, ncho, nqh)):
                            splits = [(i * ncp // nqh, (i + 1) * ncp // nqh) for i in range(nqh)]
                            for qi, (s0, s1) in enumerate(splits):
                                if s1 == s0:
                                    continue
                                nc.gpsimd.dma_gather(
                                    g[:, c0 + s0:c0 + s1, :],
                                    tabl[:, 0:roww_g] if gw else tabl[:],
                                    ix[:, s0 * 8:s1 * 8],
                                    (s1 - s0) * P, (s1 - s0) * P, roww_g,
                                    elem_step=roww,
                                    single_packet=spkt,
                                    queue_num=min(q0 + qi, nq - 1))

                    # one-hot M (no gather dependency — keeps DVE fed)
                    m = wrk.tile([P, G * T, P], BF16, tag="m")
                    nc.vector.tensor_tensor(
                        out=m[:, 0:ncht, :],
                        in0=iota[:, None, :].to_broadcast([P, ncht, P]),
                        in1=dslot_sb[:, (g0 * T):(g0 * T) + ncht, None].to_broadcast([P, ncht, P]),
                        op=mybir.AluOpType.is_equal)
                    # transposed one-hot m2[k, ch, e] = (k == dslot[ch, e])
                    dsTr = wrk1.tile([P, G * T * P], DST_DT, tag="dsTr")
                    nc.sync.dma_start(dsTr[:, 0:ncht * P], dslotTr_in[:, g0 * T * P:(g0 * T + ncht) * P])
                    m2 = wrk.tile([P, G * T, P], BF16, tag="m2")
                    m2f = m2[:].rearrange("p c e -> p (c e)")
                    m2_eng = nc.gpsimd if cfg.get("gpoff", 0) else nc.vector
                    m2_eng.tensor_tensor(out=m2f[:, 0:ncht * P], in0=dsTr[:, 0:ncht * P],
                                         in1=iotaT[:, 0, None].to_broadcast([P, ncht * P]),
                                         op=mybir.AluOpType.is_equal)
                    # per-edge s_dst via PE broadcast: sdps[e, ch, h]
                    sdps = None
                    if not cfg.get("skip_sdmm"):
                        sdps = ps.tile([P, G * T, HEADS], F32, tag="sdps")
                        for b in range(g0, g1):
                            bi = b - g0
                            chunks = [bi * te + j for j in range(te)] + [nche + bi * to + j for j in range(to)]
                            for ch in chunks:
                                nc.tensor.matmul(sdps[:, ch, 0:nh], m2[:, ch, :], sdst_cur[:, b, 0:nh],
                                                 start=True, stop=True)
                    return dict(g=g, m=m, sdps=sdps)

                def stage_b1(gi, st):
                    g0, g1 = groups[gi]
                    gn = g1 - g0
                    nche = gn * te
                    ncho = gn * to
                    ncht = nche + ncho
                    g, m, sdps = st["g"], st["m"], st["sdps"]
                    gw = bool(cfg.get("gw256")) and l < 2
                    g32 = g[:].bitcast(F32)
                    so2 = 60 if gw else soff // 2 + nh     # gw: garbage cols, timing-only
                    ssrc = g32[:, 0:ncht, so2: so2 + nh]
                    e_t = wrk.tile([P, G * T, nh], F32, tag="e_t")
                    if sdps is None:
                        nc.vector.tensor_copy(e_t[:, 0:ncht, :], ssrc)
                    else:
                        nc.vector.tensor_tensor(out=e_t[:, 0:ncht, :], in0=sdps[:, 0:ncht, 0:nh], in1=ssrc, op=mybir.AluOpType.add)
                    lr = wrk.tile([P, G * T, nh], F32, tag="lr")
                    nc.vector.scalar_tensor_tensor(
                        out=lr[:, 0:ncht, :], in0=e_t[:, 0:ncht, :], scalar=0.2,
                        in1=e_t[:, 0:ncht, :],
                        op0=mybir.AluOpType.mult, op1=mybir.AluOpType.max)
                    ee = wrk.tile([P, G * T, nh], BF16, tag="ee")
                    nc.scalar.activation(ee[:, 0:ncht, :], lr[:, 0:ncht, :], mybir.ActivationFunctionType.Exp)

                    # scale gathered features by per-head ee (in place)
                    fw = HID if gw else HID + 1
                    gv = g[:, 0:ncht, 0:nh * fw].rearrange("p c (h f) -> p c h f", h=nh)
                    nc.vector.tensor_tensor(
                        out=gv, in0=gv,
                        in1=ee[:, 0:ncht, :, None].to_broadcast([P, ncht, nh, fw]),
                        op=mybir.AluOpType.mult)

                    # aggregate per block into PSUM, copy out to group tile
                    nw = nh * fw if gw else nch_w
                    gacc = wrk.tile([P, G, nch_w], F32, tag="gacc")
                    for b in range(g0, g1):
                        bi = b - g0
                        acc = ps.tile([P, nch_w], F32, tag="agg")
                        chunks = [bi * te + j for j in range(te)] + [nche + bi * to + j for j in range(to)]
                        for ci, ch in enumerate(chunks):
                            nc.tensor.matmul(acc[:, 0:nw], m[:, ch, :], g[:, ch, 0:nw],
                                             start=(ci == 0), stop=(ci == len(chunks) - 1))
                        cp_eng = nc.gpsimd if cfg.get("gpoff", 0) else nc.vector
                        cp_eng.tensor_copy(gacc[:, bi, :], acc[:])
                    st["gacc"] = gacc

                def stage_b2(gi, st):
                    g0, g1 = groups[gi]
                    gn = g1 - g0
                    pack = pk.tile([P, G, TBLW], BF16, tag="packL")
                    if l == 2:
                        pack_y = pk.tile([P, G, OUTD], F32, tag="packy")
                    else:
                        pack_y = None
                    finalize_group(l, st["gacc"], gn, g0, pack, sdst_nxt, pack_y)
                    if l < 2:
                        nl = l + 1
                        nhn = NH[nl]
                        on = pack[:, 0:gn, 0:nhn * (HID + 1)].rearrange("p g (h c) -> p g h c", h=nhn)[:, :, :, HID:HID + 1]
                        nc.vector.memset(on, 1.0)
                        for par, stg in ((0, stg_e[nl]), (1, stg_o[nl])):
                            nc.sync.dma_start(
                                stg[g0 * 64:(g0 + gn) * 64, :].rearrange("(g q) w -> q g w", g=gn),
                                pack[par::2, 0:gn, 0:CW[nl]])
                        if gi in end_group_slice:
                            ag_slice(nl, end_group_slice[gi])
                    else:
                        nc.sync.dma_start(
                            y_out[g0 * P:g1 * P, :].rearrange("(g q) w -> q g w", g=gn),
                            pack_y[:, 0:gn, :])

                ng = len(groups)
                sts = {0: stage_a(0)}
                if ng > 1:
                    sts[1] = stage_a(1)
                stage_b1(0, sts[0])
                for gi in range(ng):
                    if gi + 2 < ng:
                        sts[gi + 2] = stage_a(gi + 2)
                    if gi + 1 < ng:
                        stage_b1(gi + 1, sts[gi + 1])
                    stage_b2(gi, sts[gi])
                    del sts[gi]

        for rep in range(reps):
            rep_body()

    nc.compile()
    return nc


# ----------------------------------------------------------------------------
# Host wrapper
# ----------------------------------------------------------------------------

_CACHE = {}
_PRE_CACHE = {}


def _scatter_A(a_s, a_d):
    nh, hc = a_s.shape
    A = np.zeros((nh * hc, 2 * nh), np.float32)
    for hd in range(nh):
        A[hd * hc:(hd + 1) * hc, hd] = a_s[hd]
        A[hd * hc:(hd + 1) * hc, nh + hd] = a_d[hd]
    return A


def prepare_in_maps(inputs, nblk, te, to, G, n_nodes, nslice=1, dst8=1):
    T = te + to
    nloc = nblk * P
    npair_c = nloc // 2
    npair = NCORES * npair_c
    groups = [(i, min(i + G, nblk)) for i in range(0, nblk, G)]
    edge_index = np.asarray(inputs["edge_index"])
    x = np.asarray(inputs["x"], np.float32)

    # slice-major table row map (must match build_kernel's slicing)
    cuts = [round(i * len(groups) / nslice) for i in range(nslice + 1)]
    blk_b0 = np.zeros(nblk, np.int64)
    blk_b1 = np.zeros(nblk, np.int64)
    for s in range(nslice):
        if cuts[s + 1] > cuts[s]:
            sb0 = groups[cuts[s]][0]
            sb1 = groups[cuts[s + 1] - 1][1]
            blk_b0[sb0:sb1] = sb0
            blk_b1[sb0:sb1] = sb1
    q = np.arange(npair)
    qc = q // npair_c
    lp = q % npair_c
    qb = lp // 64
    rowmap = 512 * blk_b0[qb] + qc * 64 * (blk_b1[qb] - blk_b0[qb]) + (lp - 64 * blk_b0[qb])

    pkey = (nblk, te, to, hash(edge_index.tobytes()))
    if pkey not in _PRE_CACHE:
        _PRE_CACHE[pkey] = preprocess(edge_index, n_nodes, nblk, te, to)
    new_id, plans = _PRE_CACHE[pkey]

    # full x in new-id order, pre-transposed, bf16 (replicated to all cores)
    xs_full = np.zeros((NCORES * nloc, F_IN), np.float32)
    xs_full[new_id] = x
    xT_full = np.ascontiguousarray(xs_full.T.astype(BF))

    # common inputs
    iota = np.tile(np.arange(P, dtype=BF), (P, 1))
    ident = np.eye(P, dtype=np.float32)
    Wl = {f"W{l}": np.asarray(inputs[f"W{l}"], np.float32) for l in range(3)}
    Al = {f"A{l}": _scatter_A(np.asarray(inputs[f"as{l}"], np.float32),
                              np.asarray(inputs[f"ad{l}"], np.float32)) for l in range(3)}
    bnl = {f"bn{l}": np.stack([np.asarray(inputs[k + str(l)], np.float32)
                               for k in ("b", "g", "bt", "m", "v")]) for l in range(3)}

    in_maps = []
    for c in range(NCORES):
        pl = plans[c]
        xTo = np.ascontiguousarray(xT_full[:, c * nloc:(c + 1) * nloc])
        # streams in tile order
        fe_l, fo_l, dsl_l = [], [], []
        for g0, g1 in groups:
            blks = list(range(g0, g1))
            ie = np.concatenate([pl["idx_e"][b] for b in blks])
            io = np.concatenate([pl["idx_o"][b] for b in blks])
            de = np.concatenate([pl["ds_e"][b] for b in blks])
            do = np.concatenate([pl["ds_o"][b] for b in blks])
            fe_l.append(np.where(ie >= 0, rowmap[np.maximum(ie, 0) >> 1], npair))
            fo_l.append(np.where(io >= 0, rowmap[np.maximum(io, 0) >> 1], npair))
            dsl_l.append(np.concatenate([de, do]))
        idx_e = _wrap_idx(np.concatenate(fe_l))
        idx_o = _wrap_idx(np.concatenate(fo_l))
        dsl_cat = np.concatenate(dsl_l)
        dslot = np.ascontiguousarray(dsl_cat.reshape(-1, P).T.astype(BF))
        ds_dt = np.int8 if dst8 else BF
        dslotTr = np.ascontiguousarray(
            np.broadcast_to(dsl_cat.astype(ds_dt)[None, :], (P, dsl_cat.size)))
        im = dict(xT=xT_full, xTo=xTo, idx_e=idx_e, idx_o=idx_o,
                  dslot=dslot, dslotTr=dslotTr,
                  Wc=np.asarray(inputs["Wc"], np.float32),
                  bc=np.asarray(inputs["bc"], np.float32).reshape(1, OUTD),
                  ident=ident, iota=iota)
        im.update(Wl)
        im.update(Al)
        im.update(bnl)
        in_maps.append(im)
    return new_id, in_maps


def run(inputs, nblk, te, to, G, n_nodes, trace=False, noop=False, nslice=1):
    new_id, in_maps = prepare_in_maps(inputs, nblk, te, to, G, n_nodes, nslice=nslice)

    import os
    skip_ag = bool(int(os.environ.get("K_SKIP_AG", "0")))
    skip_gather = bool(int(os.environ.get("K_SKIP_GATHER", "0")))
    key = (nblk, te, to, G, skip_ag, skip_gather, noop, nslice)
    if key not in _CACHE:
        _CACHE[key] = build_kernel(dict(nblk=nblk, te=te, to=to, g=G, n_nodes=n_nodes,
                                        skip_ag=skip_ag, skip_gather=skip_gather,
                                        noop=noop, nslice=nslice))
    nc = _CACHE[key]

    import time
    res = run_bass_kernel_spmd(nc, in_maps, core_ids=list(range(NCORES)), trace=False)
    if trace:
        # warm timing runs: wall-clock of the PJRT execute path
        walls = []
        for _ in range(3):
            t0 = time.perf_counter()
            res = run_bass_kernel_spmd(nc, in_maps, core_ids=list(range(NCORES)), trace=False)
            walls.append(time.perf_counter() - t0)
        res.exec_time_ns = int(min(walls) * 1e9)
    y_cat = np.concatenate([r["y"] for r in res.results])  # [NCORES*nloc, 2]
    return y_cat[new_id], res


def kernel(**inputs) -> np.ndarray:
    out, _ = run(inputs, nblk=49, te=9, to=9, G=3, n_nodes=N_FULL)
    return out.astype(np.float32)
